# revision 2
# baseline (speedup 1.0000x reference)
"""Trainium2 Bass kernel v2 for batched nearest-neighbor min-distance.

Problem: for each row u of U_z [16384, 256], compute
    min_{l in L_z [8192, 256]} ||u - l||_2
Sharding: data-parallel over U rows across 8 cores; L_z replicated.
`pred` is unused by the reference.

v2 design (per core, 2048 U rows, 64 L-tiles of 128 rows):
  d2(u,l) = ||u||^2 + ||l||^2 - 2 u.l
  - Matmuls in fp8e4m3 with perf_mode=DoubleRow: one matmul contracts
    K=256 (two interleaved K=128 halves) with free dim 512.
  - Work unit is a PSUM half-tile [128 Lrows, 1024 Ucols] (2 banks,
    ring of 4 = all 8 banks, PE runs ~2 L-tiles ahead). Each half is
    consumed by exactly ONE engine op (this removes all running-min
    merge traffic, the bottleneck of the previous design):
      'D' halves: DVE scalar_tensor_tensor folds (psum + (l2-C)) with a
          running min into rmin2 (fp16, SBUF) straight from PSUM.
      'A' halves: ACT converts psum + (l2-C) -> fp16 tile, which the
          otherwise-idle DMA engines ship to DRAM; the host min-folds
          the shipped tiles during the unshard (the 128-partition fold
          happens on host anyway).
    The A:D ratio balances ACT (1x @1.2GHz) vs DVE (1x @0.96GHz).
  - No on-device partition reduction: host folds 128 L-lanes, adds
    ||u||^2 + C, clamps, sqrts.
The C=256 shift keeps fp16 intermediates centered near 0.
"""

import numpy as np

N, M, D = 16384, 8192, 256
CORES = 8
C_SHIFT = 256.0

_COMPILED = {}

# Per-half engine assignment: 'A' = ACT-convert + DMA out, 'D' = DVE stt
# running-min. n_act/nhalves balances ACT (1038ns/half) vs DVE (1192ns/half);
# the last tail_a halves are forced to 'A' so the rmin2 output DMAs overlap
# the final ACT stretch.
N_ACT_FRAC = 68 / 128
TAIL_A = 4


def _assignment(nhalves: int, frac: float = N_ACT_FRAC, tail_a: int = TAIL_A):
    n_act = round(nhalves * frac)
    body = nhalves - tail_a
    n_act_body = n_act - tail_a
    out = []
    acc = 0
    for h in range(body):
        acc += n_act_body
        if acc >= body:
            acc -= body
            out.append("A")
        else:
            out.append("D")
    out.extend("A" * tail_a)
    assert sum(1 for a in out if a == "A") == n_act
    return out


def _build(ucols: int, m: int, debug: bool = False, rounds: int = 1,
           frac: float = N_ACT_FRAC, tail_a: int = TAIL_A, conv_bufs: int = 3,
           warmup_mms: int = 4, conv_group: int = 4, pool_memset: bool = True,
           bench_small_out: bool = False, ship: bool = True):
    from contextlib import ExitStack, nullcontext

    import concourse.bacc as bacc
    import concourse.tile as tile
    from concourse import mybir

    F32 = mybir.dt.float32
    F16 = mybir.dt.float16
    F8 = mybir.dt.float8e4
    AF = mybir.ActivationFunctionType
    ALU = mybir.AluOpType
    DR = mybir.MatmulPerfMode.DoubleRow

    ltiles = m // 128
    half = ucols // 2
    nhalves = 2 * ltiles
    assign = _assignment(nhalves, frac, tail_a)
    n_act = sum(1 for a in assign if a == "A")
    assert ucols % 1024 == 0 and m % 128 == 0

    nc = bacc.Bacc("TRN2", target_bir_lowering=False, debug=debug)

    ut_d = nc.dram_tensor("ut", [128, 2 * ucols], F8, kind="ExternalInput").ap()
    lt_d = nc.dram_tensor("lt", [128, ltiles * 2 * 128], F8,
                          kind="ExternalInput").ap()
    l2c_d = nc.dram_tensor("l2c", [128, ltiles], F32, kind="ExternalInput").ap()
    # Shipped ACT halves land at [:, j, :]; host folds them.
    # bench_small_out: alias all conv-group DMAs into an 8-slot dram region —
    # identical device-side DMA sizes/counts, but a tiny PJRT output transfer
    # (the full 17MB acts output swamps wall-clock timing over axon).
    acts_slots = 8 if bench_small_out else max(n_act, 1)
    acts_d = nc.dram_tensor("acts", [128, acts_slots, half], F16,
                            kind="ExternalOutput").ap()
    rmin_d = nc.dram_tensor("rmin0", [128, ucols], F16,
                            kind="ExternalOutput").ap()

    with tile.TileContext(nc) as tc, ExitStack() as ctx:
        const_pool = ctx.enter_context(tc.tile_pool(name="const", bufs=1))
        # Dedicated 2-deep PSUM rings per consumer engine so neither engine's
        # refill latency leaks into the other's period.
        psum_a = ctx.enter_context(
            tc.tile_pool(name="psuma", bufs=2, space="PSUM"))
        psum_d = ctx.enter_context(
            tc.tile_pool(name="psumd", bufs=2, space="PSUM"))
        conv_pool = ctx.enter_context(tc.tile_pool(name="conv", bufs=conv_bufs))

        ut_sb = const_pool.tile([128, 2 * ucols], F8, name="utsb")
        lt_sb = const_pool.tile([128, ltiles * 2 * 128], F8, name="ltsb")
        l2c = const_pool.tile([128, ltiles], F32, name="l2c")
        # Ping-pong running-min buffers (avoids DVE WAW sems between
        # consecutive stt ops); host folds both.
        rmin2 = [const_pool.tile([128, ucols], F16, name=f"rmin2{k}")
                 for k in range(2)]
        wsrc = const_pool.tile([128, 512], F8, name="wsrc")

        dummy16 = const_pool.tile([1, 2], F16, name="dummy16")

        loop_cm = tc.For_i(0, rounds, 1) if rounds > 1 else nullcontext()
        ctx.enter_context(loop_cm)

        # Trigger the (lazily inserted) ACT table load immediately so it
        # doesn't sit in front of the first real conversion.
        nc.vector.memset(dummy16[:], 0.0)
        nc.scalar.activation(dummy16[:], dummy16[:], AF.Identity)

        if warmup_mms:
            # Dummy matmuls during the DMA head keep the PE HAM clock warm.
            nc.vector.memset(wsrc[:], 1.0)
            wpsum = psum_a.tile([128, half], F32, name="psuma", tag="psuma")
            for _ in range(warmup_mms):
                nc.tensor.matmul(wpsum[:, :512], wsrc[:, :128], wsrc[:],
                                 start=True, stop=True)

        for k in range(2):
            if pool_memset:
                nc.gpsimd.memset(rmin2[k][:], 60000.0)
            else:
                nc.vector.memset(rmin2[k][:], 60000.0)
        # Input DMA order minimizes time-to-first-compute: the first A-half
        # (L-tile 0, cols 0:1024) needs only the small ut pieces + 2 L-tiles
        # + l2c (~0.5MB); everything else streams behind.
        utd_v = ut_d.rearrange("p (j n) -> p j n", j=2)
        ut_sb_v = ut_sb.rearrange("p (j n) -> p j n", j=2)
        for j in range(2):
            nc.sync.dma_start(ut_sb_v[:, j, 0:1024], utd_v[:, j, 0:1024])
        CW0 = 2 * 256
        nc.sync.dma_start(lt_sb[:, 0:CW0], lt_d[:, 0:CW0])
        nc.sync.dma_start(l2c[:], l2c_d[:])
        for j in range(2):
            nc.sync.dma_start(ut_sb_v[:, j, 1024:ucols], utd_v[:, j, 1024:ucols])
        TPC = 8  # L-tiles per input DMA chunk
        CW = TPC * 256
        for c0 in range(CW0, ltiles * 256, CW):
            cw = min(CW, ltiles * 256 - c0)
            nc.sync.dma_start(lt_sb[:, c0:c0 + cw], lt_d[:, c0:c0 + cw])

        utv = ut_sb.rearrange("p (j n) -> p j n", j=2)
        ltv = lt_sb.rearrange("p (t j m) -> p t j m", t=ltiles, j=2)

        # conv-group plan: bulk groups of conv_group, groups of 2 near the end.
        group_of = {}
        i = 0
        while i < n_act:
            rem = n_act - i
            gsz = conv_group if rem > 8 else (2 if rem > 4 else 1)
            gsz = min(gsz, rem)
            for k in range(gsz):
                group_of[i + k] = (i, gsz)
            i += gsz

        act_idx = 0
        d_idx = 0
        conv = None
        for h in range(nhalves):
            t, side = h // 2, h % 2
            bias = l2c[:, t:t + 1]
            if assign[h] == "A":
                ps = psum_a.tile([128, half], F32, name="psuma", tag="psuma")
            else:
                ps = psum_d.tile([128, half], F32, name="psumd", tag="psumd")
            for c in range(0, half, 512):
                nc.tensor.matmul(
                    ps[:, c:c + 512],
                    ltv[:, t],
                    utv[:, :, side * half + c:side * half + c + 512],
                    start=True, stop=True, perf_mode=DR,
                )
            if assign[h] == "A":
                j0, gsz = group_of[act_idx]
                g = act_idx - j0
                if g == 0:
                    conv = conv_pool.tile([128, conv_group * half], F16,
                                          name="conv", tag="conv")
                nc.scalar.activation(conv[:, g * half:(g + 1) * half], ps[:],
                                     AF.Identity, bias=bias, scale=1.0)
                if ship and g == gsz - 1:
                    jd = (j0 % 8) if bench_small_out else j0
                    nc.sync.dma_start(acts_d[:, jd:jd + gsz],
                                      conv[:, :gsz * half])
                act_idx += 1
            else:
                dst = rmin2[d_idx % 2][:, side * half:(side + 1) * half]
                nc.vector.scalar_tensor_tensor(
                    dst, ps[:], bias, dst, op0=ALU.add, op1=ALU.min)
                d_idx += 1
        assert act_idx == n_act

        # Fold the ping-pong running mins on the (now idle) DVE and ship from
        # the DVE's own DMA queue (SP is still draining conv groups).
        nc.vector.tensor_tensor(rmin2[0][:], rmin2[0][:], rmin2[1][:],
                                op=ALU.min)
        nc.scalar.dma_start(rmin_d[:], rmin2[0][:])

    nc.compile()
    return nc


def _get_compiled(ucols: int, m: int):
    key = (ucols, m)
    if key not in _COMPILED:
        _COMPILED[key] = _build(ucols, m)
    return _COMPILED[key]


def _prep_inputs(U: np.ndarray, L: np.ndarray):
    """Host-side sharding / layout prep (transpose, -2 scale, norms, fp8)."""
    import ml_dtypes

    n, d = U.shape
    m = L.shape[0]
    ucols = n // CORES
    ltiles = m // 128
    UTm2 = np.clip((-2.0 * U).T, -240.0, 240.0).astype(ml_dtypes.float8_e4m3)
    UT8 = np.ascontiguousarray(UTm2.reshape(2, 128, n).transpose(1, 0, 2))
    LT = np.clip(L.T, -240.0, 240.0).astype(ml_dtypes.float8_e4m3)
    LT8 = np.ascontiguousarray(
        LT.reshape(2, 128, ltiles, 128).transpose(1, 2, 0, 3)
    ).reshape(128, ltiles * 2 * 128)
    l2 = (L.astype(np.float64) ** 2).sum(1).astype(np.float32)
    l2cT = np.ascontiguousarray((l2 - C_SHIFT).reshape(ltiles, 128).T)
    in_maps = []
    for i in range(CORES):
        sl = slice(i * ucols, (i + 1) * ucols)
        in_maps.append({
            "ut": np.ascontiguousarray(UT8[:, :, sl]).reshape(128, 2 * ucols),
            "lt": LT8,
            "l2c": l2cT,
        })
    return in_maps


def kernel(**inputs) -> np.ndarray:
    from concourse import bass_utils

    U = np.asarray(inputs["U_z"], dtype=np.float32)
    L = np.asarray(inputs["L_z"], dtype=np.float32)
    n = U.shape[0]
    m = L.shape[0]
    ucols = n // CORES
    half = ucols // 2
    nc = _get_compiled(ucols, m)
    in_maps = _prep_inputs(U, L)
    res = bass_utils.run_bass_kernel_spmd(nc, in_maps, list(range(CORES)))

    assign = _assignment(2 * (m // 128))
    act_sides = [h % 2 for h in range(len(assign)) if assign[h] == "A"]
    u2 = (U.astype(np.float64) ** 2).sum(1).astype(np.float32)
    outs = []
    for i, r in enumerate(res.results):
        # Start from the DVE-maintained running min, fold in shipped ACT halves.
        pmin = r["rmin0"].astype(np.float32).min(axis=0)  # [ucols]
        acts = r["acts"].astype(np.float32)  # [128, n_act, half]
        fold = acts.min(axis=0)  # [n_act, half]
        for j, side in enumerate(act_sides):
            sl = slice(side * half, (side + 1) * half)
            pmin[sl] = np.minimum(pmin[sl], fold[j])
        d2 = u2[i * ucols:(i + 1) * ucols] + C_SHIFT + pmin
        outs.append(np.sqrt(np.maximum(d2, 0.0)))
    return np.concatenate(outs).astype(np.float32)


if __name__ == "__main__":
    rng = np.random.default_rng(0)
    U = rng.standard_normal((N, D), dtype=np.float32)
    L = rng.standard_normal((M, D), dtype=np.float32)
    out = kernel(pred=None, U_z=U, L_z=L)
    d2 = (U * U).sum(1)[:, None] + (L * L).sum(1)[None, :] - 2.0 * U @ L.T
    exp = np.sqrt(np.maximum(d2, 0.0).min(1))
    rel = np.abs(out - exp) / np.maximum(np.abs(exp), 1e-9)
    print("max rel err:", rel.max(), " mean:", rel.mean())


# revision 13
# speedup vs baseline: 1.8494x; 1.8494x over previous
"""Trainium2 Bass kernel v2 for batched nearest-neighbor min-distance.

Problem: for each row u of U_z [16384, 256], compute
    min_{l in L_z [8192, 256]} ||u - l||_2
Sharding: data-parallel over U rows across 8 cores; L_z replicated.
`pred` is unused by the reference.

v2 design (per core, 2048 U rows, 64 L-tiles of 128 rows):
  d2(u,l) = ||u||^2 + ||l||^2 - 2 u.l
  - Matmuls in fp8e4m3 with perf_mode=DoubleRow: one matmul contracts
    K=256 (two interleaved K=128 halves) with free dim 512.
  - Work unit is a PSUM half-tile [128 Lrows, 1024 Ucols] (2 banks,
    ring of 4 = all 8 banks, PE runs ~2 L-tiles ahead). Each half is
    consumed by exactly ONE engine op (this removes all running-min
    merge traffic, the bottleneck of the previous design):
      'D' halves: DVE scalar_tensor_tensor folds (psum + (l2-C)) with a
          running min into rmin2 (fp16, SBUF) straight from PSUM.
      'A' halves: ACT converts psum + (l2-C) -> fp16 tile, which the
          otherwise-idle DMA engines ship to DRAM; the host min-folds
          the shipped tiles during the unshard (the 128-partition fold
          happens on host anyway).
    The A:D ratio balances ACT (1x @1.2GHz) vs DVE (1x @0.96GHz).
  - No on-device partition reduction: host folds 128 L-lanes, adds
    ||u||^2 + C, clamps, sqrts.
The C=256 shift keeps fp16 intermediates centered near 0.
"""

import numpy as np

N, M, D = 16384, 8192, 256
CORES = 8
C_SHIFT = 256.0

_COMPILED = {}

# Per-half engine assignment: 'A' = ACT-convert + DMA out, 'D' = DVE stt
# running-min. n_act/nhalves balances ACT (1038ns/half) vs DVE (1192ns/half);
# the last tail_a halves are forced to 'A' so the rmin2 output DMAs overlap
# the final ACT stretch.
N_ACT_FRAC = 68 / 128
TAIL_A = 4
# The last TAIL_MERGE ACT halves can be min-merged on the idle DVE at the end
# instead of being shipped; measured slower in sim (serial end-merges cost
# more than the saved DMA drain), so disabled.
TAIL_MERGE = 0


def _assignment(nhalves: int, frac: float = N_ACT_FRAC, tail_a: int = TAIL_A):
    n_act = round(nhalves * frac)
    body = nhalves - tail_a
    n_act_body = n_act - tail_a
    out = []
    acc = 0
    for h in range(body):
        acc += n_act_body
        if acc >= body:
            acc -= body
            out.append("A")
        else:
            out.append("D")
    out.extend("A" * tail_a)
    assert sum(1 for a in out if a == "A") == n_act
    return out


def _build(ucols: int, m: int, debug: bool = False, rounds: int = 1,
           frac: float = N_ACT_FRAC, tail_a: int = TAIL_A, conv_bufs: int = 4,
           warmup_mms: int = 4, conv_group: int = 4, pool_memset: bool = True,
           bench_small_out: bool = False, ship: bool = True,
           tail_merge: int = TAIL_MERGE):
    from contextlib import ExitStack, nullcontext

    import concourse.bacc as bacc
    import concourse.tile as tile
    from concourse import mybir

    F32 = mybir.dt.float32
    F16 = mybir.dt.float16
    F8 = mybir.dt.float8e4
    AF = mybir.ActivationFunctionType
    ALU = mybir.AluOpType
    DR = mybir.MatmulPerfMode.DoubleRow

    ltiles = m // 128
    half = ucols // 2
    nhalves = 2 * ltiles
    assign = _assignment(nhalves, frac, tail_a)
    n_act = sum(1 for a in assign if a == "A")
    assert ucols % 1024 == 0 and m % 128 == 0

    nc = bacc.Bacc("TRN2", target_bir_lowering=False, debug=debug)

    ut_d = nc.dram_tensor("ut", [128, 2 * ucols], F8, kind="ExternalInput").ap()
    lt_d = nc.dram_tensor("lt", [128, ltiles * 2 * 128], F8,
                          kind="ExternalInput").ap()
    l2c_d = nc.dram_tensor("l2c", [128, ltiles], F32, kind="ExternalInput").ap()
    # Shipped ACT halves land at [:, j, :]; host folds them.
    # bench_small_out: alias all conv-group DMAs into an 8-slot dram region —
    # identical device-side DMA sizes/counts, but a tiny PJRT output transfer
    # (the full 17MB acts output swamps wall-clock timing over axon).
    n_ship = max(n_act - tail_merge, 0)
    acts_slots = 8 if bench_small_out else max(n_ship, 1)
    acts_d = nc.dram_tensor("acts", [128, acts_slots, half], F16,
                            kind="ExternalOutput").ap()
    rmin_d = nc.dram_tensor("rmin0", [128, ucols], F16,
                            kind="ExternalOutput").ap()

    with tile.TileContext(nc) as tc, ExitStack() as ctx:
        const_pool = ctx.enter_context(tc.tile_pool(name="const", bufs=1))
        # Dedicated 2-deep PSUM rings per consumer engine so neither engine's
        # refill latency leaks into the other's period.
        psum_a = ctx.enter_context(
            tc.tile_pool(name="psuma", bufs=2, space="PSUM"))
        psum_d = ctx.enter_context(
            tc.tile_pool(name="psumd", bufs=2, space="PSUM"))
        conv_pool = ctx.enter_context(tc.tile_pool(name="conv", bufs=conv_bufs))

        ut_sb = const_pool.tile([128, 2 * ucols], F8, name="utsb")
        lt_sb = const_pool.tile([128, ltiles * 2 * 128], F8, name="ltsb")
        l2c = const_pool.tile([128, ltiles], F32, name="l2c")
        # Ping-pong running-min buffers (avoids DVE WAW sems between
        # consecutive stt ops); host folds both.
        rmin2 = [const_pool.tile([128, ucols], F16, name=f"rmin2{k}")
                 for k in range(2)]
        wsrc = const_pool.tile([128, 512], F8, name="wsrc")

        dummy16 = const_pool.tile([1, 2], F16, name="dummy16")

        loop_cm = tc.For_i(0, rounds, 1) if rounds > 1 else nullcontext()
        ctx.enter_context(loop_cm)

        # Trigger the (lazily inserted) ACT table load immediately so it
        # doesn't sit in front of the first real conversion.
        nc.vector.memset(dummy16[:], 0.0)
        nc.scalar.activation(dummy16[:], dummy16[:], AF.Identity)

        if warmup_mms:
            # Dummy matmuls during the DMA head keep the PE HAM clock warm.
            nc.vector.memset(wsrc[:], 1.0)
            wpsum = psum_a.tile([128, half], F32, name="psuma", tag="psuma")
            for _ in range(warmup_mms):
                nc.tensor.matmul(wpsum[:, :512], wsrc[:, :128], wsrc[:],
                                 start=True, stop=True)

        for k in range(2):
            if pool_memset:
                nc.gpsimd.memset(rmin2[k][:], 60000.0)
            else:
                nc.vector.memset(rmin2[k][:], 60000.0)
        # Input DMA order minimizes time-to-first-compute: the first A-half
        # (L-tile 0, cols 0:1024) needs only the small ut pieces + 2 L-tiles
        # + l2c (~0.5MB); everything else streams behind.
        utd_v = ut_d.rearrange("p (j n) -> p j n", j=2)
        ut_sb_v = ut_sb.rearrange("p (j n) -> p j n", j=2)
        CW0 = 2 * 256
        nc.sync.dma_start(ut_sb_v[:, 0, 0:1024], utd_v[:, 0, 0:1024])
        nc.sync.dma_start(lt_sb[:, 0:CW0], lt_d[:, 0:CW0])
        nc.sync.dma_start(ut_sb_v[:, 1, 0:1024], utd_v[:, 1, 0:1024])
        nc.sync.dma_start(l2c[:], l2c_d[:])
        for j in range(2):
            nc.sync.dma_start(ut_sb_v[:, j, 1024:ucols], utd_v[:, j, 1024:ucols])
        TPC = 8  # L-tiles per input DMA chunk
        CW = TPC * 256
        for c0 in range(CW0, ltiles * 256, CW):
            cw = min(CW, ltiles * 256 - c0)
            nc.sync.dma_start(lt_sb[:, c0:c0 + cw], lt_d[:, c0:c0 + cw])

        utv = ut_sb.rearrange("p (j n) -> p j n", j=2)
        ltv = lt_sb.rearrange("p (t j m) -> p t j m", t=ltiles, j=2)

        # conv-group plan: bulk groups of conv_group, groups of 2 near the end.
        group_of = {}
        i = 0
        while i < n_ship:
            rem = n_ship - i
            gsz = conv_group if rem > 8 else (2 if rem > 4 else 1)
            gsz = min(gsz, rem)
            for k in range(gsz):
                group_of[i + k] = (i, gsz)
            i += gsz

        n_d = nhalves - n_act
        act_idx = 0
        d_idx = 0
        conv = None
        tail_convs = []
        for h in range(nhalves):
            t, side = h // 2, h % 2
            bias = l2c[:, t:t + 1]
            if assign[h] == "A":
                ps = psum_a.tile([128, half], F32, name="psuma", tag="psuma")
            else:
                ps = psum_d.tile([128, half], F32, name="psumd", tag="psumd")
            for c in range(0, half, 512):
                nc.tensor.matmul(
                    ps[:, c:c + 512],
                    ltv[:, t],
                    utv[:, :, side * half + c:side * half + c + 512],
                    start=True, stop=True, perf_mode=DR,
                )
            if assign[h] == "A":
                if act_idx >= n_ship:
                    tconv = conv_pool.tile([128, conv_group * half], F16,
                                           name="conv", tag="conv")
                    nc.scalar.activation(tconv[:, :half], ps[:],
                                         AF.Identity, bias=bias, scale=1.0)
                    tail_convs.append((tconv[:, :half], side))
                    act_idx += 1
                    continue
                j0, gsz = group_of[act_idx]
                g = act_idx - j0
                if g == 0:
                    conv = conv_pool.tile([128, conv_group * half], F16,
                                          name="conv", tag="conv")
                nc.scalar.activation(conv[:, g * half:(g + 1) * half], ps[:],
                                     AF.Identity, bias=bias, scale=1.0)
                if ship and g == gsz - 1:
                    jd = (j0 % 8) if bench_small_out else j0
                    nc.sync.dma_start(acts_d[:, jd:jd + gsz],
                                      conv[:, :gsz * half])
                act_idx += 1
            else:
                dst = rmin2[d_idx % 2][:, side * half:(side + 1) * half]
                nc.vector.scalar_tensor_tensor(
                    dst, ps[:], bias, dst, op0=ALU.add, op1=ALU.min)
                d_idx += 1
        assert act_idx == n_act

        # Fold the ping-pong running mins, merge any unshipped tail convs on
        # the now-idle DVE, and ship from the ACT HWDGE queue (SP is still
        # draining conv groups).
        nc.vector.tensor_tensor(rmin2[0][:], rmin2[0][:], rmin2[1][:],
                                op=ALU.min)
        for pconv, pside in tail_convs:
            dst = rmin2[0][:, pside * half:(pside + 1) * half]
            nc.vector.tensor_tensor(dst, dst, pconv, op=ALU.min)
        nc.scalar.dma_start(rmin_d[:], rmin2[0][:])

    nc.compile()
    return nc


def _get_compiled(ucols: int, m: int):
    key = (ucols, m)
    if key not in _COMPILED:
        _COMPILED[key] = _build(ucols, m)
    return _COMPILED[key]


def _prep_inputs(U: np.ndarray, L: np.ndarray):
    """Host-side sharding / layout prep (transpose, -2 scale, norms, fp8)."""
    import ml_dtypes

    n, d = U.shape
    m = L.shape[0]
    ucols = n // CORES
    ltiles = m // 128
    UTm2 = np.clip((-2.0 * U).T, -240.0, 240.0).astype(ml_dtypes.float8_e4m3)
    UT8 = np.ascontiguousarray(UTm2.reshape(2, 128, n).transpose(1, 0, 2))
    LT = np.clip(L.T, -240.0, 240.0).astype(ml_dtypes.float8_e4m3)
    LT8 = np.ascontiguousarray(
        LT.reshape(2, 128, ltiles, 128).transpose(1, 2, 0, 3)
    ).reshape(128, ltiles * 2 * 128)
    l2 = (L.astype(np.float64) ** 2).sum(1).astype(np.float32)
    l2cT = np.ascontiguousarray((l2 - C_SHIFT).reshape(ltiles, 128).T)
    in_maps = []
    for i in range(CORES):
        sl = slice(i * ucols, (i + 1) * ucols)
        in_maps.append({
            "ut": np.ascontiguousarray(UT8[:, :, sl]).reshape(128, 2 * ucols),
            "lt": LT8,
            "l2c": l2cT,
        })
    return in_maps


def kernel(**inputs) -> np.ndarray:
    from concourse import bass_utils

    U = np.asarray(inputs["U_z"], dtype=np.float32)
    L = np.asarray(inputs["L_z"], dtype=np.float32)
    n = U.shape[0]
    m = L.shape[0]
    ucols = n // CORES
    half = ucols // 2
    nc = _get_compiled(ucols, m)
    in_maps = _prep_inputs(U, L)
    res = bass_utils.run_bass_kernel_spmd(nc, in_maps, list(range(CORES)))

    assign = _assignment(2 * (m // 128))
    act_sides = [h % 2 for h in range(len(assign)) if assign[h] == "A"]
    u2 = (U.astype(np.float64) ** 2).sum(1).astype(np.float32)
    outs = []
    for i, r in enumerate(res.results):
        # Start from the DVE-maintained running min, fold in shipped ACT halves.
        pmin = r["rmin0"].astype(np.float32).min(axis=0)  # [ucols]
        acts = r["acts"].astype(np.float32)  # [128, n_ship, half]
        fold = acts.min(axis=0)  # [n_ship, half]
        for j, side in enumerate(act_sides[:acts.shape[1]]):
            sl = slice(side * half, (side + 1) * half)
            pmin[sl] = np.minimum(pmin[sl], fold[j])
        d2 = u2[i * ucols:(i + 1) * ucols] + C_SHIFT + pmin
        outs.append(np.sqrt(np.maximum(d2, 0.0)))
    return np.concatenate(outs).astype(np.float32)


if __name__ == "__main__":
    rng = np.random.default_rng(0)
    U = rng.standard_normal((N, D), dtype=np.float32)
    L = rng.standard_normal((M, D), dtype=np.float32)
    out = kernel(pred=None, U_z=U, L_z=L)
    d2 = (U * U).sum(1)[:, None] + (L * L).sum(1)[None, :] - 2.0 * U @ L.T
    exp = np.sqrt(np.maximum(d2, 0.0).min(1))
    rel = np.abs(out - exp) / np.maximum(np.abs(exp), 1e-9)
    print("max rel err:", rel.max(), " mean:", rel.mean())
